# revision 12
# baseline (speedup 1.0000x reference)
"""Trainium2 Bass kernel for nn_DiffusionBlock: 20 steps of a 5-point
reflect-padded diffusion stencil on (16, 1, 1024, 1024) fp32.

The step operator is linear/separable and diagonalized analytically by the
DCT-I basis v_k[i] = cos(pi*k*i/(N-1)); the T-step result is the spectral
map Y = F [ M * (E^T X E) ] F^T with M_ij = (a + lv_i + lw_j)^T.

Eigenvector parity (v_k[N-1-i] = (-1)^k v_k[i]) folds X into 4 parity
quadrants (512x512) with 4 independent half-size spectral pipelines.

v2 on top of the folded pipeline:
  - Spectral truncation + packing: per quadrant dim, only modes with
    |M| >= ~6e-4 survive 20 steps -- 186 low + 186 high of 512.  They are
    permuted into a contiguous [192 low | 192 high-reversed] block of 384
    (3 tiles of 128); the dead middle 128 modes are never computed.
    S1/S5 shrink 25%, S2/S4 keep only the 7 live 128x128 mode-pair blocks
    (low-low + high-high disks; max dropped |M| ~ 6e-4, adds ~1e-3 err).
  - Full bf16 data path (same 1 cyc/row PE rate as fp32r, half the DMA
    and drain traffic); PSUM accumulation stays fp32; mask stays fp32.
  - PSUM drains alternate Scalar/Vector engines; mask-multiply on Vector.

Data-parallel over batch: 2 images/core on 8 NeuronCores.
"""

import sys

import ml_dtypes
import numpy as np

if "/opt/trn_rl_repo" not in sys.path:
    sys.path.insert(0, "/opt/trn_rl_repo")

import concourse.bass as bass  # noqa: E402
import concourse.tile as tile  # noqa: E402
from concourse import bacc, mybir  # noqa: E402
from concourse.bass_utils import run_bass_kernel_spmd  # noqa: E402

N = 1024
H = 512           # half size after parity fold
P = 128
KCH = H // P      # 4 space chunks per quadrant
NL = 192          # live modes kept per (low, high) corner
LIVE = 2 * NL     # 384 = 3 tiles of 128
RT = LIVE // P    # 3 mode tiles
NCORES = 8
IMGS_PER_CORE = 2

XQW = KCH * H          # 2048 cols per quadrant of folded X (chunked by i)
WQW = KCH * LIVE       # 1536 cols per quadrant of W      (chunked by j2)
GQW = RT * LIVE        # 1152 cols per quadrant of G      (chunked by r)
HQW = RT * H           # 1536 cols per quadrant of H      (chunked by s)
YQW = KCH * H          # 2048 cols per quadrant of Y      (chunked by i')

# S2 computed column range per r-tile; S4 live r-chunks per s-tile.
# Valid because mode-pair liveness is two diagonal corner disks (verified:
# max |M| outside this pattern < 6e-11).
COLR = [(0, 256), (0, 384), (128, 384)]
MOFF = [0, 256, 640]
MQW = 896              # mask cols stored per quadrant
LIVE_KK = {0: (0, 1), 1: (0, 1, 2), 2: (1, 2)}

RHO = [0, 0, 1, 1]
GAM = [0, 1, 0, 1]

_BASS_CACHE = {}
_MAT_CACHE = {}

BF16 = ml_dtypes.bfloat16


def _chunk(a, p=P):
    """(R, C) -> (p, (R//p)*C); chunk k holds rows [p*k, p*(k+1))."""
    r, c = a.shape
    return np.ascontiguousarray(
        a.reshape(r // p, p, c).transpose(1, 0, 2).reshape(p, (r // p) * c))


def _unchunk(t, rows):
    p, w = t.shape
    c = w // (rows // p)
    return np.ascontiguousarray(
        t.reshape(p, rows // p, c).transpose(1, 0, 2).reshape(rows, c))


def _build_specs(weight, time_steps):
    key = (weight.tobytes(), int(time_steps))
    if key in _MAT_CACHE:
        return _MAT_CACHE[key]
    w = np.asarray(weight, dtype=np.float64).reshape(3, 3)
    assert max(abs(w[0, 0]), abs(w[0, 2]), abs(w[2, 0]), abs(w[2, 2])) < 1e-12
    assert abs(w[0, 1] - w[2, 1]) < 1e-12 and abs(w[1, 0] - w[1, 2]) < 1e-12
    a_c = w[1, 1]
    k = np.arange(N)
    i = np.arange(N)
    lam = np.cos(np.pi * k / (N - 1))
    V = np.cos(np.pi * np.outer(i, k) / (N - 1))
    d = np.ones(N)
    d[0] = 0.5
    d[-1] = 0.5
    wn = np.sqrt((d[:, None] * V * V).sum(axis=0))
    E = (d[:, None] * V) / wn[None, :]
    Fm = V / wn[None, :]
    lv = (w[0, 1] + w[2, 1]) * lam
    lw = (w[1, 0] + w[1, 2]) * lam
    M = (a_c + lv[:, None] + lw[None, :]) ** int(time_steps)

    perm = np.concatenate([np.arange(NL), np.arange(H - 1, H - 1 - NL, -1)])
    eh = np.concatenate(
        [_chunk(np.ascontiguousarray(E[:H, p::2][:, perm])) for p in (0, 1)],
        axis=1).astype(BF16)                      # [P, 2*WQW]
    fh = np.concatenate(
        [_chunk(np.ascontiguousarray(Fm[:H, p::2][:, perm].T)) for p in (0, 1)],
        axis=1).astype(BF16)                      # [P, 2*HQW]
    mq = np.zeros((P, 4 * MQW), dtype=np.float32)
    for q in range(4):
        Mq = M[RHO[q]::2, :][:, GAM[q]::2][np.ix_(perm, perm)]
        for m in range(RT):
            lo, hi = COLR[m]
            mq[:, q * MQW + MOFF[m]: q * MQW + MOFF[m] + (hi - lo)] = \
                Mq[P * m: P * (m + 1), lo:hi]
    out = (eh, fh, mq.astype(np.float32))
    _MAT_CACHE[key] = out
    return out


def _fold_image(img):
    """(1024, 1024) f32 -> (P, 4*XQW) bf16 quadrant-folded chunk layout."""
    a = img.astype(np.float32)
    xp = a[:H] + a[N - 1:H - 1:-1]
    xm = a[:H] - a[N - 1:H - 1:-1]
    qs = []
    for xr in (xp, xm):
        qs.append(xr[:, :H] + xr[:, N - 1:H - 1:-1])
        qs.append(xr[:, :H] - xr[:, N - 1:H - 1:-1])
    return np.concatenate([_chunk(q) for q in qs], axis=1).astype(BF16)


def _unfold_image(yq):
    """(P, 4*YQW) bf16 quadrant outputs -> (1024, 1024) f32."""
    Qs = [_unchunk(yq[:, YQW * q: YQW * (q + 1)].astype(np.float32), H)
          for q in range(4)]
    Ypp, Ypm, Ymp, Ymm = [q.astype(np.float64) for q in Qs]
    Y = np.empty((N, N), dtype=np.float32)
    Y[:H, :H] = Ypp + Ypm + Ymp + Ymm
    Y[:H, H:] = (Ypp - Ypm + Ymp - Ymm)[:, ::-1]
    Y[H:, :H] = (Ypp + Ypm - Ymp - Ymm)[::-1, :]
    Y[H:, H:] = (Ypp - Ypm - Ymp + Ymm)[::-1, ::-1]
    return Y


def _build_bass():
    if "nc" in _BASS_CACHE:
        return _BASS_CACHE["nc"]
    nc = bacc.Bacc("TRN2", target_bir_lowering=False, debug=False,
                   num_devices=NCORES)
    f32 = mybir.dt.float32
    b16 = mybir.dt.bfloat16
    xq_d = nc.dram_tensor("xq", [IMGS_PER_CORE, P, 4 * XQW], b16,
                          kind="ExternalInput").ap()
    eh_d = nc.dram_tensor("eh", [P, 2 * WQW], b16, kind="ExternalInput").ap()
    fh_d = nc.dram_tensor("fh", [P, 2 * HQW], b16, kind="ExternalInput").ap()
    mq_d = nc.dram_tensor("mq", [P, 4 * MQW], f32, kind="ExternalInput").ap()
    yq_d = nc.dram_tensor("yq", [IMGS_PER_CORE, P, 4 * YQW], b16,
                          kind="ExternalOutput").ap()

    drain_flip = [0]

    def drain(nc, out_ap, ps_ap):
        # alternate ACT/DVE so neither engine becomes the drain bottleneck
        if drain_flip[0] & 1:
            nc.vector.tensor_copy(out=out_ap, in_=ps_ap)
        else:
            nc.scalar.copy(out=out_ap, in_=ps_ap)
        drain_flip[0] += 1

    def drain_split(nc, out_ap, ps_ap, w):
        # half on ACT + half on DVE concurrently: ~halves drain latency
        nc.scalar.copy(out=out_ap[:, 0:w // 2], in_=ps_ap[:, 0:w // 2])
        nc.vector.tensor_copy(out=out_ap[:, w // 2:w], in_=ps_ap[:, w // 2:w])

    with tile.TileContext(nc) as tc:
        with tc.tile_pool(name="const", bufs=1) as cpool, \
             tc.tile_pool(name="data", bufs=2) as dpool, \
             tc.tile_pool(name="psum", bufs=8, space="PSUM") as ppool:
            eh_t = cpool.tile([P, 2 * WQW], b16, tag="eh")
            fh_t = cpool.tile([P, 2 * HQW], b16, tag="fh")
            mq_t = cpool.tile([P, 4 * MQW], f32, tag="mq")

            xas = [dpool.tile([P, 4 * XQW], b16, tag="xa", name=f"xa{i}")
                   for i in range(IMGS_PER_CORE)]
            # Input DMA.  Each dma_start costs ~0.7us of descriptor build on
            # the issuing engine's queue, so (a) issue from two engines in
            # parallel (sync: eh+X q0/q2, scalar: eh p1 + X q1/q3), (b) pin
            # high priority so the scheduler can't defer any issue behind
            # later work.  mq/fh follow: first needed at S2/S4.
            with tc.high_priority():
                nc.sync.dma_start(out=eh_t[:, 0:1536], in_=eh_d[:, 0:1536])
                nc.scalar.dma_start(out=eh_t[:, 1536:3072],
                                    in_=eh_d[:, 1536:3072])
                nc.sync.dma_start(out=xas[0][:, 0:2048], in_=xq_d[0, :, 0:2048])
                nc.scalar.dma_start(out=xas[0][:, 2048:4096],
                                    in_=xq_d[0, :, 2048:4096])
                nc.sync.dma_start(out=xas[0][:, 4096:6144],
                                  in_=xq_d[0, :, 4096:6144])
                nc.scalar.dma_start(out=xas[0][:, 6144:8192],
                                    in_=xq_d[0, :, 6144:8192])
                nc.sync.dma_start(out=mq_t[:, 0:1792], in_=mq_d[:, 0:1792])
                nc.scalar.dma_start(out=mq_t[:, 1792:3584],
                                    in_=mq_d[:, 1792:3584])
                nc.sync.dma_start(out=fh_t[:, 0:1536], in_=fh_d[:, 0:1536])
                nc.scalar.dma_start(out=fh_t[:, 1536:3072],
                                    in_=fh_d[:, 1536:3072])

            for img in range(IMGS_PER_CORE):
                xa = xas[img]
                wb = dpool.tile([P, 4 * WQW], b16, tag="wb")
                for q in range(4):      # S1: W = X^T E_rho  (live cols only)
                    rb = WQW * RHO[q]
                    # kk-major: 4 concurrent PSUM groups, so each arriving
                    # xa/eh DMA piece immediately unblocks 4 matmuls instead
                    # of the whole quadrant gating the first group
                    pss = [ppool.tile([P, H], mybir.dt.float32, tag="ps",
                                      name=f"ps_s1_{img}_{q}_{m}")
                           for m in range(KCH)]
                    for kk in range(KCH):
                        for m in range(KCH):
                            nc.tensor.matmul(
                                out=pss[m][:, 0:LIVE],
                                lhsT=xa[:, XQW * q + H * kk + P * m:
                                        XQW * q + H * kk + P * (m + 1)],
                                rhs=eh_t[:, rb + LIVE * kk:
                                         rb + LIVE * (kk + 1)],
                                start=(kk == 0), stop=(kk == KCH - 1))
                    for m in range(KCH):
                        drain(nc, wb[:, WQW * q + LIVE * m:
                                     WQW * q + LIVE * (m + 1)],
                              pss[m][:, 0:LIVE])

                if img + 1 < IMGS_PER_CORE:
                    # queue next image's input behind the constants
                    for c in range(4):
                        s = slice(2048 * c, 2048 * (c + 1))
                        nc.sync.dma_start(out=xas[img + 1][:, s],
                                          in_=xq_d[img + 1, :, s])

                ga = dpool.tile([P, 4 * GQW], b16, tag="ga")
                for q in range(4):      # S2+S3: G = M * (W^T E_gam), 7 blocks
                    gb = WQW * GAM[q]
                    for m in range(RT):
                        lo, hi = COLR[m]
                        ps = ppool.tile([P, H], mybir.dt.float32, tag="ps")
                        for kk in range(KCH):
                            nc.tensor.matmul(
                                out=ps[:, 0:hi - lo],
                                lhsT=wb[:, WQW * q + LIVE * kk + P * m:
                                        WQW * q + LIVE * kk + P * (m + 1)],
                                rhs=eh_t[:, gb + LIVE * kk + lo:
                                         gb + LIVE * kk + hi],
                                start=(kk == 0), stop=(kk == KCH - 1))
                        nc.vector.tensor_tensor(
                            out=ga[:, GQW * q + LIVE * m + lo:
                                   GQW * q + LIVE * m + hi],
                            in0=ps[:, 0:hi - lo],
                            in1=mq_t[:, MQW * q + MOFF[m]:
                                     MQW * q + MOFF[m] + (hi - lo)],
                            op=mybir.AluOpType.mult)

                hb = dpool.tile([P, 4 * HQW], b16, tag="hb")
                yc = dpool.tile([P, 4 * YQW], b16, tag="yc")
                # S4 then S5 per quadrant: spreads output DMA over the whole
                # second half instead of crowding it behind the last matmuls
                for q in range(4):
                    fb = HQW * RHO[q]
                    for m in range(RT):  # S4: H = G^T Ft_rho (live chunks)
                        live = LIVE_KK[m]
                        ps = ppool.tile([P, H], mybir.dt.float32, tag="ps")
                        for kk in live:
                            nc.tensor.matmul(
                                out=ps[:, 0:H],
                                lhsT=ga[:, GQW * q + LIVE * kk + P * m:
                                        GQW * q + LIVE * kk + P * (m + 1)],
                                rhs=fh_t[:, fb + H * kk: fb + H * (kk + 1)],
                                start=(kk == live[0]), stop=(kk == live[-1]))
                        drain(nc, hb[:, HQW * q + H * m: HQW * q + H * (m + 1)],
                              ps[:, 0:H])
                    fb = HQW * GAM[q]
                    for m in range(KCH):  # S5: Y = H^T Ft_gam
                        ps = ppool.tile([P, H], mybir.dt.float32, tag="ps")
                        for kk in range(RT):
                            nc.tensor.matmul(
                                out=ps[:, 0:H],
                                lhsT=hb[:, HQW * q + H * kk + P * m:
                                        HQW * q + H * kk + P * (m + 1)],
                                rhs=fh_t[:, fb + H * kk: fb + H * (kk + 1)],
                                start=(kk == 0), stop=(kk == RT - 1))
                        s = slice(YQW * q + H * m, YQW * q + H * (m + 1))
                        drain_split(nc, yc[:, s], ps[:, 0:H], H)
                        nc.sync.dma_start(out=yq_d[img, :, s], in_=yc[:, s])

    nc.compile()
    _BASS_CACHE["nc"] = nc
    return nc


def kernel(x, weight, time_steps, **_ignored):
    x = np.asarray(x, dtype=np.float32)
    weight = np.asarray(weight, dtype=np.float32)
    eh, fh, mq = _build_specs(weight, time_steps)
    nc = _build_bass()

    b = x.shape[0]
    assert b == NCORES * IMGS_PER_CORE and x.shape[-2:] == (N, N)
    in_maps = []
    for c in range(NCORES):
        xq = np.stack([_fold_image(x[c * IMGS_PER_CORE + i, 0])
                       for i in range(IMGS_PER_CORE)])
        in_maps.append({"xq": xq, "eh": eh, "fh": fh, "mq": mq})

    res = run_bass_kernel_spmd(nc, in_maps, core_ids=list(range(NCORES)))
    _BASS_CACHE["last_results"] = res

    out = np.empty((b, 1, N, N), dtype=np.float32)
    for c in range(NCORES):
        ys = res.results[c]["yq"]
        for i in range(IMGS_PER_CORE):
            out[c * IMGS_PER_CORE + i, 0] = _unfold_image(ys[i])
    return out


# revision 13
# speedup vs baseline: 1.0134x; 1.0134x over previous
"""Trainium2 Bass kernel for nn_DiffusionBlock: 20 steps of a 5-point
reflect-padded diffusion stencil on (16, 1, 1024, 1024) fp32.

The step operator is linear/separable and diagonalized analytically by the
DCT-I basis v_k[i] = cos(pi*k*i/(N-1)); the T-step result is the spectral
map Y = F [ M * (E^T X E) ] F^T with M_ij = (a + lv_i + lw_j)^T.

Eigenvector parity (v_k[N-1-i] = (-1)^k v_k[i]) folds X into 4 parity
quadrants (512x512) with 4 independent half-size spectral pipelines.

v2 on top of the folded pipeline:
  - Spectral truncation + packing: per quadrant dim, only modes with
    |M| >= ~6e-4 survive 20 steps -- 186 low + 186 high of 512.  They are
    permuted into a contiguous [192 low | 192 high-reversed] block of 384
    (3 tiles of 128); the dead middle 128 modes are never computed.
    S1/S5 shrink 25%, S2/S4 keep only the 7 live 128x128 mode-pair blocks
    (low-low + high-high disks; max dropped |M| ~ 6e-4, adds ~1e-3 err).
  - Full bf16 data path (same 1 cyc/row PE rate as fp32r, half the DMA
    and drain traffic); PSUM accumulation stays fp32; mask stays fp32.
  - PSUM drains alternate Scalar/Vector engines; mask-multiply on Vector.

Data-parallel over batch: 2 images/core on 8 NeuronCores.
"""

import sys

import ml_dtypes
import numpy as np

if "/opt/trn_rl_repo" not in sys.path:
    sys.path.insert(0, "/opt/trn_rl_repo")

import concourse.bass as bass  # noqa: E402
import concourse.tile as tile  # noqa: E402
from concourse import bacc, mybir  # noqa: E402
from concourse.bass_utils import run_bass_kernel_spmd  # noqa: E402

N = 1024
H = 512           # half size after parity fold
P = 128
KCH = H // P      # 4 space chunks per quadrant
NL = 192          # live modes kept per (low, high) corner
LIVE = 2 * NL     # 384 = 3 tiles of 128
RT = LIVE // P    # 3 mode tiles
NCORES = 8
IMGS_PER_CORE = 2

XQW = KCH * H          # 2048 cols per quadrant of folded X (chunked by i)
WQW = KCH * LIVE       # 1536 cols per quadrant of W      (chunked by j2)
GQW = RT * LIVE        # 1152 cols per quadrant of G      (chunked by r)
HQW = RT * H           # 1536 cols per quadrant of H      (chunked by s)
YQW = KCH * H          # 2048 cols per quadrant of Y      (chunked by i')

# S2 computed column range per r-tile; S4 live r-chunks per s-tile.
# Valid because mode-pair liveness is two diagonal corner disks (verified:
# max |M| outside this pattern < 6e-11).
COLR = [(0, 256), (0, 384), (128, 384)]
MOFF = [0, 256, 640]
MQW = 896              # mask cols stored per quadrant
LIVE_KK = {0: (0, 1), 1: (0, 1, 2), 2: (1, 2)}

RHO = [0, 0, 1, 1]
GAM = [0, 1, 0, 1]

_BASS_CACHE = {}
_MAT_CACHE = {}

BF16 = ml_dtypes.bfloat16


def _chunk(a, p=P):
    """(R, C) -> (p, (R//p)*C); chunk k holds rows [p*k, p*(k+1))."""
    r, c = a.shape
    return np.ascontiguousarray(
        a.reshape(r // p, p, c).transpose(1, 0, 2).reshape(p, (r // p) * c))


def _unchunk(t, rows):
    p, w = t.shape
    c = w // (rows // p)
    return np.ascontiguousarray(
        t.reshape(p, rows // p, c).transpose(1, 0, 2).reshape(rows, c))


def _build_specs(weight, time_steps):
    key = (weight.tobytes(), int(time_steps))
    if key in _MAT_CACHE:
        return _MAT_CACHE[key]
    w = np.asarray(weight, dtype=np.float64).reshape(3, 3)
    assert max(abs(w[0, 0]), abs(w[0, 2]), abs(w[2, 0]), abs(w[2, 2])) < 1e-12
    assert abs(w[0, 1] - w[2, 1]) < 1e-12 and abs(w[1, 0] - w[1, 2]) < 1e-12
    a_c = w[1, 1]
    k = np.arange(N)
    i = np.arange(N)
    lam = np.cos(np.pi * k / (N - 1))
    V = np.cos(np.pi * np.outer(i, k) / (N - 1))
    d = np.ones(N)
    d[0] = 0.5
    d[-1] = 0.5
    wn = np.sqrt((d[:, None] * V * V).sum(axis=0))
    E = (d[:, None] * V) / wn[None, :]
    Fm = V / wn[None, :]
    lv = (w[0, 1] + w[2, 1]) * lam
    lw = (w[1, 0] + w[1, 2]) * lam
    M = (a_c + lv[:, None] + lw[None, :]) ** int(time_steps)

    perm = np.concatenate([np.arange(NL), np.arange(H - 1, H - 1 - NL, -1)])
    eh = np.concatenate(
        [_chunk(np.ascontiguousarray(E[:H, p::2][:, perm])) for p in (0, 1)],
        axis=1).astype(BF16)                      # [P, 2*WQW]
    fh = np.concatenate(
        [_chunk(np.ascontiguousarray(Fm[:H, p::2][:, perm].T)) for p in (0, 1)],
        axis=1).astype(BF16)                      # [P, 2*HQW]
    mq = np.zeros((P, 4 * MQW), dtype=np.float32)
    for q in range(4):
        Mq = M[RHO[q]::2, :][:, GAM[q]::2][np.ix_(perm, perm)]
        for m in range(RT):
            lo, hi = COLR[m]
            mq[:, q * MQW + MOFF[m]: q * MQW + MOFF[m] + (hi - lo)] = \
                Mq[P * m: P * (m + 1), lo:hi]
    out = (eh, fh, mq.astype(np.float32))
    _MAT_CACHE[key] = out
    return out


def _fold_image(img):
    """(1024, 1024) f32 -> (P, 4*XQW) bf16 quadrant-folded chunk layout."""
    a = img.astype(np.float32)
    xp = a[:H] + a[N - 1:H - 1:-1]
    xm = a[:H] - a[N - 1:H - 1:-1]
    qs = []
    for xr in (xp, xm):
        qs.append(xr[:, :H] + xr[:, N - 1:H - 1:-1])
        qs.append(xr[:, :H] - xr[:, N - 1:H - 1:-1])
    return np.concatenate([_chunk(q) for q in qs], axis=1).astype(BF16)


def _unfold_image(yq):
    """(P, 4*YQW) bf16 quadrant outputs -> (1024, 1024) f32."""
    Qs = [_unchunk(yq[:, YQW * q: YQW * (q + 1)].astype(np.float32), H)
          for q in range(4)]
    Ypp, Ypm, Ymp, Ymm = [q.astype(np.float64) for q in Qs]
    Y = np.empty((N, N), dtype=np.float32)
    Y[:H, :H] = Ypp + Ypm + Ymp + Ymm
    Y[:H, H:] = (Ypp - Ypm + Ymp - Ymm)[:, ::-1]
    Y[H:, :H] = (Ypp + Ypm - Ymp - Ymm)[::-1, :]
    Y[H:, H:] = (Ypp - Ypm - Ymp + Ymm)[::-1, ::-1]
    return Y


def _build_bass():
    if "nc" in _BASS_CACHE:
        return _BASS_CACHE["nc"]
    nc = bacc.Bacc("TRN2", target_bir_lowering=False, debug=False,
                   num_devices=NCORES)
    f32 = mybir.dt.float32
    b16 = mybir.dt.bfloat16
    xq_d = nc.dram_tensor("xq", [IMGS_PER_CORE, P, 4 * XQW], b16,
                          kind="ExternalInput").ap()
    eh_d = nc.dram_tensor("eh", [P, 2 * WQW], b16, kind="ExternalInput").ap()
    fh_d = nc.dram_tensor("fh", [P, 2 * HQW], b16, kind="ExternalInput").ap()
    mq_d = nc.dram_tensor("mq", [P, 4 * MQW], f32, kind="ExternalInput").ap()
    yq_d = nc.dram_tensor("yq", [IMGS_PER_CORE, P, 4 * YQW], b16,
                          kind="ExternalOutput").ap()

    drain_flip = [0]

    def drain(nc, out_ap, ps_ap):
        # alternate ACT/DVE so neither engine becomes the drain bottleneck
        if drain_flip[0] & 1:
            nc.vector.tensor_copy(out=out_ap, in_=ps_ap)
        else:
            nc.scalar.copy(out=out_ap, in_=ps_ap)
        drain_flip[0] += 1

    def drain_split(nc, out_ap, ps_ap, w):
        # half on ACT + half on DVE concurrently: ~halves drain latency
        nc.scalar.copy(out=out_ap[:, 0:w // 2], in_=ps_ap[:, 0:w // 2])
        nc.vector.tensor_copy(out=out_ap[:, w // 2:w], in_=ps_ap[:, w // 2:w])

    with tile.TileContext(nc) as tc:
        with tc.tile_pool(name="const", bufs=1) as cpool, \
             tc.tile_pool(name="data", bufs=2) as dpool, \
             tc.tile_pool(name="psum", bufs=8, space="PSUM") as ppool:
            eh_t = cpool.tile([P, 2 * WQW], b16, tag="eh")
            fh_t = cpool.tile([P, 2 * HQW], b16, tag="fh")
            mq_t = cpool.tile([P, 4 * MQW], f32, tag="mq")

            xas = [dpool.tile([P, 4 * XQW], b16, tag="xa", name=f"xa{i}")
                   for i in range(IMGS_PER_CORE)]
            # Input DMA, high priority so the scheduler cannot defer any
            # issue (each dma_start costs ~0.7us of descriptor build and a
            # deferred issue stalls the PE).  Sync queue carries the
            # critical-path pieces in first-use order; mq/fh (needed only
            # at S2/S4) go on the scalar queue in parallel.
            with tc.high_priority():
                nc.sync.dma_start(out=eh_t[:, 0:768], in_=eh_d[:, 0:768])
                nc.sync.dma_start(out=xas[0][:, 0:1024],
                                  in_=xq_d[0, :, 0:1024])
                nc.sync.dma_start(out=eh_t[:, 768:1536], in_=eh_d[:, 768:1536])
                nc.sync.dma_start(out=xas[0][:, 1024:2048],
                                  in_=xq_d[0, :, 1024:2048])
                nc.sync.dma_start(out=xas[0][:, 2048:4096],
                                  in_=xq_d[0, :, 2048:4096])
                nc.sync.dma_start(out=eh_t[:, 1536:3072],
                                  in_=eh_d[:, 1536:3072])
                nc.sync.dma_start(out=xas[0][:, 4096:6144],
                                  in_=xq_d[0, :, 4096:6144])
                nc.sync.dma_start(out=xas[0][:, 6144:8192],
                                  in_=xq_d[0, :, 6144:8192])
                nc.scalar.dma_start(out=mq_t[:, 0:1792], in_=mq_d[:, 0:1792])
                nc.scalar.dma_start(out=mq_t[:, 1792:3584],
                                    in_=mq_d[:, 1792:3584])
                nc.scalar.dma_start(out=fh_t[:, 0:1536], in_=fh_d[:, 0:1536])
                nc.scalar.dma_start(out=fh_t[:, 1536:3072],
                                    in_=fh_d[:, 1536:3072])

            for img in range(IMGS_PER_CORE):
                xa = xas[img]
                wb = dpool.tile([P, 4 * WQW], b16, tag="wb")
                for q in range(4):      # S1: W = X^T E_rho  (live cols only)
                    rb = WQW * RHO[q]
                    # kk-major: 4 concurrent PSUM groups, so each arriving
                    # xa/eh DMA piece immediately unblocks 4 matmuls instead
                    # of the whole quadrant gating the first group
                    pss = [ppool.tile([P, H], mybir.dt.float32, tag="ps",
                                      name=f"ps_s1_{img}_{q}_{m}")
                           for m in range(KCH)]
                    for kk in range(KCH):
                        for m in range(KCH):
                            nc.tensor.matmul(
                                out=pss[m][:, 0:LIVE],
                                lhsT=xa[:, XQW * q + H * kk + P * m:
                                        XQW * q + H * kk + P * (m + 1)],
                                rhs=eh_t[:, rb + LIVE * kk:
                                         rb + LIVE * (kk + 1)],
                                start=(kk == 0), stop=(kk == KCH - 1))
                    for m in range(KCH):
                        drain(nc, wb[:, WQW * q + LIVE * m:
                                     WQW * q + LIVE * (m + 1)],
                              pss[m][:, 0:LIVE])

                if img + 1 < IMGS_PER_CORE:
                    # queue next image's input behind the constants
                    for c in range(4):
                        s = slice(2048 * c, 2048 * (c + 1))
                        nc.sync.dma_start(out=xas[img + 1][:, s],
                                          in_=xq_d[img + 1, :, s])

                ga = dpool.tile([P, 4 * GQW], b16, tag="ga")
                for q in range(4):      # S2+S3: G = M * (W^T E_gam), 7 blocks
                    gb = WQW * GAM[q]
                    for m in range(RT):
                        lo, hi = COLR[m]
                        ps = ppool.tile([P, H], mybir.dt.float32, tag="ps")
                        for kk in range(KCH):
                            nc.tensor.matmul(
                                out=ps[:, 0:hi - lo],
                                lhsT=wb[:, WQW * q + LIVE * kk + P * m:
                                        WQW * q + LIVE * kk + P * (m + 1)],
                                rhs=eh_t[:, gb + LIVE * kk + lo:
                                         gb + LIVE * kk + hi],
                                start=(kk == 0), stop=(kk == KCH - 1))
                        nc.vector.tensor_tensor(
                            out=ga[:, GQW * q + LIVE * m + lo:
                                   GQW * q + LIVE * m + hi],
                            in0=ps[:, 0:hi - lo],
                            in1=mq_t[:, MQW * q + MOFF[m]:
                                     MQW * q + MOFF[m] + (hi - lo)],
                            op=mybir.AluOpType.mult)

                hb = dpool.tile([P, 4 * HQW], b16, tag="hb")
                yc = dpool.tile([P, 4 * YQW], b16, tag="yc")
                # S4 then S5 per quadrant: spreads output DMA over the whole
                # second half instead of crowding it behind the last matmuls
                for q in range(4):
                    fb = HQW * RHO[q]
                    for m in range(RT):  # S4: H = G^T Ft_rho (live chunks)
                        live = LIVE_KK[m]
                        ps = ppool.tile([P, H], mybir.dt.float32, tag="ps")
                        for kk in live:
                            nc.tensor.matmul(
                                out=ps[:, 0:H],
                                lhsT=ga[:, GQW * q + LIVE * kk + P * m:
                                        GQW * q + LIVE * kk + P * (m + 1)],
                                rhs=fh_t[:, fb + H * kk: fb + H * (kk + 1)],
                                start=(kk == live[0]), stop=(kk == live[-1]))
                        drain(nc, hb[:, HQW * q + H * m: HQW * q + H * (m + 1)],
                              ps[:, 0:H])
                    fb = HQW * GAM[q]
                    for m in range(KCH):  # S5: Y = H^T Ft_gam
                        ps = ppool.tile([P, H], mybir.dt.float32, tag="ps")
                        for kk in range(RT):
                            nc.tensor.matmul(
                                out=ps[:, 0:H],
                                lhsT=hb[:, HQW * q + H * kk + P * m:
                                        HQW * q + H * kk + P * (m + 1)],
                                rhs=fh_t[:, fb + H * kk: fb + H * (kk + 1)],
                                start=(kk == 0), stop=(kk == RT - 1))
                        s = slice(YQW * q + H * m, YQW * q + H * (m + 1))
                        drain_split(nc, yc[:, s], ps[:, 0:H], H)
                        nc.sync.dma_start(out=yq_d[img, :, s], in_=yc[:, s])

    nc.compile()
    _BASS_CACHE["nc"] = nc
    return nc


def kernel(x, weight, time_steps, **_ignored):
    x = np.asarray(x, dtype=np.float32)
    weight = np.asarray(weight, dtype=np.float32)
    eh, fh, mq = _build_specs(weight, time_steps)
    nc = _build_bass()

    b = x.shape[0]
    assert b == NCORES * IMGS_PER_CORE and x.shape[-2:] == (N, N)
    in_maps = []
    for c in range(NCORES):
        xq = np.stack([_fold_image(x[c * IMGS_PER_CORE + i, 0])
                       for i in range(IMGS_PER_CORE)])
        in_maps.append({"xq": xq, "eh": eh, "fh": fh, "mq": mq})

    res = run_bass_kernel_spmd(nc, in_maps, core_ids=list(range(NCORES)))
    _BASS_CACHE["last_results"] = res

    out = np.empty((b, 1, N, N), dtype=np.float32)
    for c in range(NCORES):
        ys = res.results[c]["yq"]
        for i in range(IMGS_PER_CORE):
            out[c * IMGS_PER_CORE + i, 0] = _unfold_image(ys[i])
    return out


# revision 14
# speedup vs baseline: 1.0601x; 1.0461x over previous
"""Trainium2 Bass kernel for nn_DiffusionBlock: 20 steps of a 5-point
reflect-padded diffusion stencil on (16, 1, 1024, 1024) fp32.

The step operator is linear/separable and diagonalized analytically by the
DCT-I basis v_k[i] = cos(pi*k*i/(N-1)); the T-step result is the spectral
map Y = F [ M * (E^T X E) ] F^T with M_ij = (a + lv_i + lw_j)^T.

Eigenvector parity (v_k[N-1-i] = (-1)^k v_k[i]) folds X into 4 parity
quadrants (512x512) with 4 independent half-size spectral pipelines.

v2 on top of the folded pipeline:
  - Spectral truncation + packing: per quadrant dim, only modes with
    |M| >= ~6e-4 survive 20 steps -- 186 low + 186 high of 512.  They are
    permuted into a contiguous [192 low | 192 high-reversed] block of 384
    (3 tiles of 128); the dead middle 128 modes are never computed.
    S1/S5 shrink 25%, S2/S4 keep only the 7 live 128x128 mode-pair blocks
    (low-low + high-high disks; max dropped |M| ~ 6e-4, adds ~1e-3 err).
  - Full bf16 data path (same 1 cyc/row PE rate as fp32r, half the DMA
    and drain traffic); PSUM accumulation stays fp32; mask stays fp32.
  - PSUM drains alternate Scalar/Vector engines; mask-multiply on Vector.

Data-parallel over batch: 2 images/core on 8 NeuronCores.
"""

import sys

import ml_dtypes
import numpy as np

if "/opt/trn_rl_repo" not in sys.path:
    sys.path.insert(0, "/opt/trn_rl_repo")

import concourse.bass as bass  # noqa: E402
import concourse.tile as tile  # noqa: E402
from concourse import bacc, mybir  # noqa: E402
from concourse.bass_utils import run_bass_kernel_spmd  # noqa: E402

N = 1024
H = 512           # half size after parity fold
P = 128
KCH = H // P      # 4 space chunks per quadrant
NL = 192          # live modes kept per (low, high) corner
LIVE = 2 * NL     # 384 = 3 tiles of 128
RT = LIVE // P    # 3 mode tiles
NCORES = 8
IMGS_PER_CORE = 2

XQW = KCH * H          # 2048 cols per quadrant of folded X (chunked by i)
WQW = KCH * LIVE       # 1536 cols per quadrant of W      (chunked by j2)
GQW = RT * LIVE        # 1152 cols per quadrant of G      (chunked by r)
HQW = RT * H           # 1536 cols per quadrant of H      (chunked by s)
YQW = KCH * H          # 2048 cols per quadrant of Y      (chunked by i')

# S2 computed column range per r-tile; S4 live r-chunks per s-tile.
# Valid because mode-pair liveness is two diagonal corner disks (verified:
# max |M| outside this pattern < 6e-11).
COLR = [(0, 256), (0, 384), (128, 384)]
MOFF = [0, 256, 640]
MQW = 896              # mask cols stored per quadrant
LIVE_KK = {0: (0, 1), 1: (0, 1, 2), 2: (1, 2)}

RHO = [0, 0, 1, 1]
GAM = [0, 1, 0, 1]

_BASS_CACHE = {}
_MAT_CACHE = {}

BF16 = ml_dtypes.bfloat16


def _chunk(a, p=P):
    """(R, C) -> (p, (R//p)*C); chunk k holds rows [p*k, p*(k+1))."""
    r, c = a.shape
    return np.ascontiguousarray(
        a.reshape(r // p, p, c).transpose(1, 0, 2).reshape(p, (r // p) * c))


def _unchunk(t, rows):
    p, w = t.shape
    c = w // (rows // p)
    return np.ascontiguousarray(
        t.reshape(p, rows // p, c).transpose(1, 0, 2).reshape(rows, c))


def _build_specs(weight, time_steps):
    key = (weight.tobytes(), int(time_steps))
    if key in _MAT_CACHE:
        return _MAT_CACHE[key]
    w = np.asarray(weight, dtype=np.float64).reshape(3, 3)
    assert max(abs(w[0, 0]), abs(w[0, 2]), abs(w[2, 0]), abs(w[2, 2])) < 1e-12
    assert abs(w[0, 1] - w[2, 1]) < 1e-12 and abs(w[1, 0] - w[1, 2]) < 1e-12
    a_c = w[1, 1]
    k = np.arange(N)
    i = np.arange(N)
    lam = np.cos(np.pi * k / (N - 1))
    V = np.cos(np.pi * np.outer(i, k) / (N - 1))
    d = np.ones(N)
    d[0] = 0.5
    d[-1] = 0.5
    wn = np.sqrt((d[:, None] * V * V).sum(axis=0))
    E = (d[:, None] * V) / wn[None, :]
    Fm = V / wn[None, :]
    lv = (w[0, 1] + w[2, 1]) * lam
    lw = (w[1, 0] + w[1, 2]) * lam
    M = (a_c + lv[:, None] + lw[None, :]) ** int(time_steps)

    perm = np.concatenate([np.arange(NL), np.arange(H - 1, H - 1 - NL, -1)])
    eh = np.concatenate(
        [_chunk(np.ascontiguousarray(E[:H, p::2][:, perm])) for p in (0, 1)],
        axis=1).astype(BF16)                      # [P, 2*WQW]
    fh = np.concatenate(
        [_chunk(np.ascontiguousarray(Fm[:H, p::2][:, perm].T)) for p in (0, 1)],
        axis=1).astype(BF16)                      # [P, 2*HQW]
    mq = np.zeros((P, 4 * MQW), dtype=np.float32)
    for q in range(4):
        Mq = M[RHO[q]::2, :][:, GAM[q]::2][np.ix_(perm, perm)]
        for m in range(RT):
            lo, hi = COLR[m]
            mq[:, q * MQW + MOFF[m]: q * MQW + MOFF[m] + (hi - lo)] = \
                Mq[P * m: P * (m + 1), lo:hi]
    out = (eh, fh, mq.astype(np.float32))
    _MAT_CACHE[key] = out
    return out


def _fold_image(img):
    """(1024, 1024) f32 -> (P, 4*XQW) bf16 quadrant-folded chunk layout."""
    a = img.astype(np.float32)
    xp = a[:H] + a[N - 1:H - 1:-1]
    xm = a[:H] - a[N - 1:H - 1:-1]
    qs = []
    for xr in (xp, xm):
        qs.append(xr[:, :H] + xr[:, N - 1:H - 1:-1])
        qs.append(xr[:, :H] - xr[:, N - 1:H - 1:-1])
    return np.concatenate([_chunk(q) for q in qs], axis=1).astype(BF16)


def _unfold_image(yq):
    """(P, 4*YQW) bf16 quadrant outputs -> (1024, 1024) f32."""
    Qs = [_unchunk(yq[:, YQW * q: YQW * (q + 1)].astype(np.float32), H)
          for q in range(4)]
    Ypp, Ypm, Ymp, Ymm = [q.astype(np.float64) for q in Qs]
    Y = np.empty((N, N), dtype=np.float32)
    Y[:H, :H] = Ypp + Ypm + Ymp + Ymm
    Y[:H, H:] = (Ypp - Ypm + Ymp - Ymm)[:, ::-1]
    Y[H:, :H] = (Ypp + Ypm - Ymp - Ymm)[::-1, :]
    Y[H:, H:] = (Ypp - Ypm - Ymp + Ymm)[::-1, ::-1]
    return Y


def _build_bass():
    if "nc" in _BASS_CACHE:
        return _BASS_CACHE["nc"]
    nc = bacc.Bacc("TRN2", target_bir_lowering=False, debug=False,
                   num_devices=NCORES)
    f32 = mybir.dt.float32
    b16 = mybir.dt.bfloat16
    xq_d = nc.dram_tensor("xq", [IMGS_PER_CORE, P, 4 * XQW], b16,
                          kind="ExternalInput").ap()
    eh_d = nc.dram_tensor("eh", [P, 2 * WQW], b16, kind="ExternalInput").ap()
    fh_d = nc.dram_tensor("fh", [P, 2 * HQW], b16, kind="ExternalInput").ap()
    mq_d = nc.dram_tensor("mq", [P, 4 * MQW], f32, kind="ExternalInput").ap()
    yq_d = nc.dram_tensor("yq", [IMGS_PER_CORE, P, 4 * YQW], b16,
                          kind="ExternalOutput").ap()

    drain_flip = [0]

    def drain(nc, out_ap, ps_ap):
        # alternate ACT/DVE so neither engine becomes the drain bottleneck
        if drain_flip[0] & 1:
            nc.vector.tensor_copy(out=out_ap, in_=ps_ap)
        else:
            nc.scalar.copy(out=out_ap, in_=ps_ap)
        drain_flip[0] += 1

    def drain_split(nc, out_ap, ps_ap, w):
        # half on ACT + half on DVE concurrently: ~halves drain latency
        nc.scalar.copy(out=out_ap[:, 0:w // 2], in_=ps_ap[:, 0:w // 2])
        nc.vector.tensor_copy(out=out_ap[:, w // 2:w], in_=ps_ap[:, w // 2:w])

    with tile.TileContext(nc) as tc:
        with tc.tile_pool(name="const", bufs=1) as cpool, \
             tc.tile_pool(name="data", bufs=2) as dpool, \
             tc.tile_pool(name="psum", bufs=8, space="PSUM") as ppool:
            eh_t = cpool.tile([P, 2 * WQW], b16, tag="eh")
            fh_t = cpool.tile([P, 2 * HQW], b16, tag="fh")
            mq_t = cpool.tile([P, 4 * MQW], f32, tag="mq")

            xas = [dpool.tile([P, 4 * XQW], b16, tag="xa", name=f"xa{i}")
                   for i in range(IMGS_PER_CORE)]
            # Input DMA, high priority so the scheduler cannot defer any
            # issue (each dma_start costs ~0.7us of descriptor build and a
            # deferred issue stalls the PE).  Sync queue carries the
            # critical-path pieces in first-use order; mq/fh (needed only
            # at S2/S4) go on the scalar queue in parallel.
            with tc.high_priority():
                nc.sync.dma_start(out=eh_t[:, 0:768], in_=eh_d[:, 0:768])
                nc.sync.dma_start(out=xas[0][:, 0:1024],
                                  in_=xq_d[0, :, 0:1024])
                nc.sync.dma_start(out=eh_t[:, 768:1536], in_=eh_d[:, 768:1536])
                nc.sync.dma_start(out=xas[0][:, 1024:2048],
                                  in_=xq_d[0, :, 1024:2048])
                nc.sync.dma_start(out=xas[0][:, 2048:4096],
                                  in_=xq_d[0, :, 2048:4096])
                nc.sync.dma_start(out=eh_t[:, 1536:3072],
                                  in_=eh_d[:, 1536:3072])
                nc.sync.dma_start(out=xas[0][:, 4096:6144],
                                  in_=xq_d[0, :, 4096:6144])
                nc.sync.dma_start(out=xas[0][:, 6144:8192],
                                  in_=xq_d[0, :, 6144:8192])
                nc.sync.dma_start(out=mq_t[:, 0:1792], in_=mq_d[:, 0:1792])
                nc.sync.dma_start(out=mq_t[:, 1792:3584],
                                  in_=mq_d[:, 1792:3584])
                nc.sync.dma_start(out=fh_t[:, 0:1536], in_=fh_d[:, 0:1536])
                nc.sync.dma_start(out=fh_t[:, 1536:3072],
                                  in_=fh_d[:, 1536:3072])

            for img in range(IMGS_PER_CORE):
                xa = xas[img]
                wb = dpool.tile([P, 4 * WQW], b16, tag="wb")
                for q in range(4):      # S1: W = X^T E_rho  (live cols only)
                    rb = WQW * RHO[q]
                    # kk-major: 4 concurrent PSUM groups, so each arriving
                    # xa/eh DMA piece immediately unblocks 4 matmuls instead
                    # of the whole quadrant gating the first group
                    pss = [ppool.tile([P, H], mybir.dt.float32, tag="ps",
                                      name=f"ps_s1_{img}_{q}_{m}")
                           for m in range(KCH)]
                    for kk in range(KCH):
                        for m in range(KCH):
                            nc.tensor.matmul(
                                out=pss[m][:, 0:LIVE],
                                lhsT=xa[:, XQW * q + H * kk + P * m:
                                        XQW * q + H * kk + P * (m + 1)],
                                rhs=eh_t[:, rb + LIVE * kk:
                                         rb + LIVE * (kk + 1)],
                                start=(kk == 0), stop=(kk == KCH - 1))
                    for m in range(KCH):
                        drain(nc, wb[:, WQW * q + LIVE * m:
                                     WQW * q + LIVE * (m + 1)],
                              pss[m][:, 0:LIVE])

                if img + 1 < IMGS_PER_CORE:
                    # queue next image's input behind the constants
                    for c in range(4):
                        s = slice(2048 * c, 2048 * (c + 1))
                        nc.sync.dma_start(out=xas[img + 1][:, s],
                                          in_=xq_d[img + 1, :, s])

                ga = dpool.tile([P, 4 * GQW], b16, tag="ga")
                for q in range(4):      # S2+S3: G = M * (W^T E_gam), 7 blocks
                    gb = WQW * GAM[q]
                    for m in range(RT):
                        lo, hi = COLR[m]
                        ps = ppool.tile([P, H], mybir.dt.float32, tag="ps")
                        for kk in range(KCH):
                            nc.tensor.matmul(
                                out=ps[:, 0:hi - lo],
                                lhsT=wb[:, WQW * q + LIVE * kk + P * m:
                                        WQW * q + LIVE * kk + P * (m + 1)],
                                rhs=eh_t[:, gb + LIVE * kk + lo:
                                         gb + LIVE * kk + hi],
                                start=(kk == 0), stop=(kk == KCH - 1))
                        nc.vector.tensor_tensor(
                            out=ga[:, GQW * q + LIVE * m + lo:
                                   GQW * q + LIVE * m + hi],
                            in0=ps[:, 0:hi - lo],
                            in1=mq_t[:, MQW * q + MOFF[m]:
                                     MQW * q + MOFF[m] + (hi - lo)],
                            op=mybir.AluOpType.mult)

                hb = dpool.tile([P, 4 * HQW], b16, tag="hb")
                yc = dpool.tile([P, 4 * YQW], b16, tag="yc")
                # S4 then S5 per quadrant: spreads output DMA over the whole
                # second half instead of crowding it behind the last matmuls
                for q in range(4):
                    fb = HQW * RHO[q]
                    for m in range(RT):  # S4: H = G^T Ft_rho (live chunks)
                        live = LIVE_KK[m]
                        ps = ppool.tile([P, H], mybir.dt.float32, tag="ps")
                        for kk in live:
                            nc.tensor.matmul(
                                out=ps[:, 0:H],
                                lhsT=ga[:, GQW * q + LIVE * kk + P * m:
                                        GQW * q + LIVE * kk + P * (m + 1)],
                                rhs=fh_t[:, fb + H * kk: fb + H * (kk + 1)],
                                start=(kk == live[0]), stop=(kk == live[-1]))
                        drain(nc, hb[:, HQW * q + H * m: HQW * q + H * (m + 1)],
                              ps[:, 0:H])
                    fb = HQW * GAM[q]
                    for m in range(KCH):  # S5: Y = H^T Ft_gam
                        ps = ppool.tile([P, H], mybir.dt.float32, tag="ps")
                        for kk in range(RT):
                            nc.tensor.matmul(
                                out=ps[:, 0:H],
                                lhsT=hb[:, HQW * q + H * kk + P * m:
                                        HQW * q + H * kk + P * (m + 1)],
                                rhs=fh_t[:, fb + H * kk: fb + H * (kk + 1)],
                                start=(kk == 0), stop=(kk == RT - 1))
                        s = slice(YQW * q + H * m, YQW * q + H * (m + 1))
                        drain_split(nc, yc[:, s], ps[:, 0:H], H)
                        nc.sync.dma_start(out=yq_d[img, :, s], in_=yc[:, s])

    nc.compile()
    _BASS_CACHE["nc"] = nc
    return nc


def kernel(x, weight, time_steps, **_ignored):
    x = np.asarray(x, dtype=np.float32)
    weight = np.asarray(weight, dtype=np.float32)
    eh, fh, mq = _build_specs(weight, time_steps)
    nc = _build_bass()

    b = x.shape[0]
    assert b == NCORES * IMGS_PER_CORE and x.shape[-2:] == (N, N)
    in_maps = []
    for c in range(NCORES):
        xq = np.stack([_fold_image(x[c * IMGS_PER_CORE + i, 0])
                       for i in range(IMGS_PER_CORE)])
        in_maps.append({"xq": xq, "eh": eh, "fh": fh, "mq": mq})

    res = run_bass_kernel_spmd(nc, in_maps, core_ids=list(range(NCORES)))
    _BASS_CACHE["last_results"] = res

    out = np.empty((b, 1, N, N), dtype=np.float32)
    for c in range(NCORES):
        ys = res.results[c]["yq"]
        for i in range(IMGS_PER_CORE):
            out[c * IMGS_PER_CORE + i, 0] = _unfold_image(ys[i])
    return out
